# revision 65
# baseline (speedup 1.0000x reference)
"""CrossAttentionS2T Trainium2 kernel (8-core data-parallel over the BT=128
frame axis), bf16 on-chip compute.

Math (per frame of 196 tokens, D=768, H=12 heads of 64):
  s_pat = s_x[:,1:,:] + clip_pos ;  t = t_x + vmae_pos
  q = t @ Wq.T ; k,v = s_pat @ Wkv.T ; attn = softmax(SCALE * q k^T)
  out = (attn @ v) @ Wp.T
Biases are zeros per the spec; a numpy fallback preserves the contract
if nonzero biases are ever passed.

Layout/engine plan per core (16 frames, processed in pairs):
  - Weights are pre-transposed to bf16 [128, kt, rows] ON HOST and DMA'd
    straight into SBUF (no on-device weight prep).
  - Inputs DMA'd fp32 token-major [112, 2, 768]; pos-embed added on DVE
    with bf16 output (GPSIMD measured ~1us/op extra on HW -- avoid).
  - PE transposes (bf16) produce sT/tT [128d, kt, 392tok]; sT then tT
    transposes, kT GEMM blocks zipped into the tT section.
  - qT/kT GEMMs: lhsT = pre-transposed bf16 weights, rhs = tT/sT, N=392.
  - v GEMM per frame-chunk: lhsT = sT chunk, rhs = WvT, out [98tok, 768].
  - QK per (frame, ci, 2-head tile): N=196 matmuls at col 0 of each bank
    of a 2-bank [98, 1024] psum tile (matmul PSUM writes MUST be
    bank-aligned on HW); ONE exp per tile reads both banks via a
    bank-strided 3D AP ([98, 2, 196]), softmax scale fused, no
    max-subtraction (logit sigma ~0.3).
  - rowsum via ones-matmul (broadcast across partitions), reciprocal on
    DVE, normalization fused into the AV evac muls.
  - AV per-head col-packed: M=64 N=196 matmuls, head 2j -> psum rows
    0:64, head 2j+1 -> rows 64:128 (2-way col-group concurrency on HW,
    ~22us/iteration faster than the block-diagonal form); accumulated
    over the two key chunks; oT [128d, kt, 196q] bf16.
  - proj GEMM lhsT=oT, rhs=WpT, fp32 evac on ACT (EVAC=1 relieves DVE),
    one DMA per frame.
Stage scheduling (SCHED=zip, the measured-best default): deferred
AV(f-1) and proj(f-2) chunks are interleaved INTO frame f's softmax
chain and into the pair-boundary transpose section, giving the PE
independent ready work while each QK->exp->rowsum->recip chain
propagates.  NOTE an interaction: zip only wins AFTER the proj evacs
moved to ACT (EVAC=1); with DVE-heavy evacs the same interleave
measured SLOWER than the sequential SCHED=old (both keep their env
flags for re-testing).
All pools double/triple-buffered; PSUM budget: at 2x2 + ot 2 + mm 2
= 8 banks.
"""
import os
import numpy as np

FUSED_OT = os.environ.get("V_FUSED_OT", "1") == "1"
# pos-add engine: 0=DVE, 1=GPSIMD, 3=split (s on GPSIMD, t on DVE)
POOL_ADD = int(os.environ.get("V_POOL_ADD", "0"))
SWPIPE = os.environ.get("V_SWPIPE", "1") == "1"
AVPACK = os.environ.get("V_AVPACK", "1") == "1"
EXPBATCH = int(os.environ.get("V_EXPBATCH", "1"))
# stage schedule: "old" = AV(f)+proj(f-1) after softmax(f); "zip" = AV(f-1)
# and proj(f-2) chunks interleaved into softmax(f)
SCHED = os.environ.get("V_SCHED", "zip")
# 1 = move osb psum evacs from DVE to ACT
EVAC = int(os.environ.get("V_EVAC", "1"))
# 1 = move sT transpose evacs from DVE to ACT (relieves boundary DVE)
TEVAC = int(os.environ.get("V_TEVAC", "0"))
# 1 = spread DMAs over both HWDGE rings (t + out on ACT ring, s on SP)
DMAQ = int(os.environ.get("V_DMAQ", "0"))
# 1 = staggered semaphore reset in the rep loop (no full-drain barrier)
STAG = int(os.environ.get("V_STAG", "0"))
IOBUFS = int(os.environ.get("V_IOBUFS", "3"))
# ablation probes (wrong numerics, HW bench only)
SKIPATT = os.environ.get("V_SKIPATT", "0") == "1"
SKIPIO = os.environ.get("V_SKIPIO", "0") == "1"

H = 12
D = 768
HD = 64
SCALE = HD ** -0.5
T = 8
N = 196
B = 16
BT = B * T          # 128 frames
NCORES = 8
F = BT // NCORES    # 16 frames per core
KT = D // 128       # 6 k-tiles
CH = [(0, 98), (98, 98)]   # token chunks of a frame
NSPLIT = [(0, 512), (512, 256)]
RSPLIT = [(0, 512), (512, 512), (1024, 512), (1536, 512), (2048, 304)]

_CACHE = {}


def _build(n_frames, reps=1, unroll=1):
    import concourse.bacc as bacc
    import concourse.bass as bass
    import concourse.tile as tile
    from concourse import mybir
    from concourse.masks import make_identity

    f32 = mybir.dt.float32
    bf16 = mybir.dt.bfloat16
    EXP = mybir.ActivationFunctionType.Exp

    def view(ap, dims):
        return bass.AP(tensor=ap.tensor, offset=ap.offset, ap=[ap.ap[0]] + dims)

    nc = bacc.Bacc("TRN2", target_bir_lowering=False, debug=False,
                   num_devices=NCORES)

    s_d = nc.declare_dram_parameter("s", [n_frames, N + 1, D], f32, isOutput=False)
    t_d = nc.declare_dram_parameter("t", [n_frames, N, D], f32, isOutput=False)
    cpos_d = nc.declare_dram_parameter("cpos", [N, D], f32, isOutput=False)
    vpos_d = nc.declare_dram_parameter("vpos", [N, D], f32, isOutput=False)
    # host-pre-transposed bf16 weights: wT[p, j, r] = W[r, j*128+p]
    wq_d = nc.declare_dram_parameter("wqT", [128, KT, D], bf16, isOutput=False)
    wk_d = nc.declare_dram_parameter("wkT", [128, KT, D], bf16, isOutput=False)
    wv_d = nc.declare_dram_parameter("wvT", [128, KT, D], bf16, isOutput=False)
    wp_d = nc.declare_dram_parameter("wpT", [128, KT, D], bf16, isOutput=False)
    out_d = nc.declare_dram_parameter("out", [n_frames, N, D], f32, isOutput=True)

    NPAIR = (n_frames + 1) // 2

    def acol(h):
        return h * 196

    with tile.TileContext(nc) as tc:
        import contextlib
        ctx = contextlib.ExitStack()
        with ctx:
            single = ctx.enter_context(tc.tile_pool(name="single", bufs=1))
            wpool = ctx.enter_context(tc.tile_pool(name="wpool", bufs=1))
            io = ctx.enter_context(tc.tile_pool(name="io", bufs=IOBUFS))
            cvt = ctx.enter_context(tc.tile_pool(
                name="cvt", bufs=int(os.environ.get("V_CVTBUFS", "3"))))
            ost = ctx.enter_context(tc.tile_pool(name="ost", bufs=2))
            work = ctx.enter_context(tc.tile_pool(name="work", bufs=2))
            rcp = ctx.enter_context(tc.tile_pool(name="rcp", bufs=2))
            ot_ps = ctx.enter_context(tc.tile_pool(name="ot_ps", bufs=2, space="PSUM"))
            mm_ps = ctx.enter_context(tc.tile_pool(
                name="mm_ps", bufs=int(os.environ.get("V_MMBUFS", "2")),
                space="PSUM"))
            at_ps = ctx.enter_context(
                tc.tile_pool(name="at_ps", bufs=(2 if EXPBATCH else 3), space="PSUM"))

            identf = single.tile([128, 128], f32)
            make_identity(nc, identf)
            ident16 = single.tile([128, 128], bf16)
            nc.vector.tensor_copy(out=ident16, in_=identf)
            ones = single.tile([128, 128], bf16)
            nc.vector.memset(ones, 1.0)

            if SKIPIO:
                s16c = single.tile([112, 2, D], bf16, tag="s16c")
                nc.gpsimd.memset(s16c, 0.1)
                t16c = single.tile([112, 2, D], bf16, tag="t16c")
                nc.gpsimd.memset(t16c, 0.1)

            # ---- pos embeds, token-major [112, 2, 768] fp32 (28 overlap) ----
            pos_ap = [[D, 112], [84 * D, 2], [1, D]]
            cpos2_sb = single.tile([112, 2, D], f32, tag="cpos")
            nc.sync.dma_start(out=cpos2_sb,
                              in_=bass.AP(tensor=cpos_d.ap().tensor, offset=0,
                                          ap=list(pos_ap)))
            vpos2_sb = single.tile([112, 2, D], f32, tag="vpos")
            nc.sync.dma_start(out=vpos2_sb,
                              in_=bass.AP(tensor=vpos_d.ap().tensor, offset=0,
                                          ap=list(pos_ap)))

            # ---- transposed bf16 weights straight from DRAM ----
            def load_wT(dram, name):
                wt = wpool.tile([128, KT, D], bf16, tag=name)
                nc.sync.dma_start(out=wt, in_=dram.ap())
                return wt

            wqT = load_wT(wq_d, "wqT")
            wkT = load_wT(wk_d, "wkT")
            wvT = load_wT(wv_d, "wvT")
            wpT = load_wT(wp_d, "wpT")

            unroll_reps = unroll
            if reps < 0:
                unroll_reps, reps = -reps, 1
            rep_ctx = (tc.For_i(0, reps, 1, staggered_reset=bool(STAG))
                       if reps > 1 else None)
            if rep_ctx is not None:
                ctx.enter_context(rep_ctx)

            PROJ_CHUNKS = [(ci, n0, nl) for ci in range(2) for n0, nl in NSPLIT]

            def emit_pj_chunk(p):
                """One proj GEMM chunk; the last chunk also emits the DMA."""
                oT, f, cb, osb = p["oT"], p["f"], p["cb"], p["osb"]
                ci, n0, nl = PROJ_CHUNKS[p["i"]]
                o, l = CH[ci]
                ps = mm_ps.tile([128, 512], f32, tag="mm")
                for kt in range(KT):
                    nc.tensor.matmul(
                        ps[:l, :nl],
                        oT[:, kt, cb + o:cb + o + l],
                        wpT[:, kt, n0:n0 + nl],
                        start=(kt == 0), stop=(kt == KT - 1))
                if EVAC:
                    nc.scalar.copy(out=osb[:, ci, n0:n0 + nl],
                                   in_=ps[:l, :nl])
                else:
                    nc.vector.tensor_copy(out=osb[:, ci, n0:n0 + nl],
                                          in_=ps[:l, :nl])
                p["i"] += 1
                if p["i"] == len(PROJ_CHUNKS):
                    (nc.scalar if DMAQ else nc.sync).dma_start(
                        out=out_d[f, :, :].rearrange("(c p) d -> p c d", p=98),
                        in_=osb)

            def emit_av_block(a):
                """AV matmuls + fused normalize-evac for one k-tile j."""
                attn_c, v_sb, rc, f = a["st"]
                oT, j = a["oT"], a["j"]
                if AVPACK:
                    # per-head M=64 matmuls; head 2j -> psum rows 0:64,
                    # head 2j+1 -> rows 64:128 (col-group concurrency);
                    # accumulate over key chunks.
                    ps = ot_ps.tile([128, 196], f32, tag="ot")
                    for hp in range(2):
                        h = 2 * j + hp
                        for ci, (ko, kl) in enumerate(CH):
                            nc.tensor.matmul(
                                ps[hp * 64:hp * 64 + 64, 0:196],
                                v_sb[ci][:kl, h * 64:h * 64 + 64],
                                attn_c[ci][:kl, acol(h):acol(h) + 196],
                                start=(ci == 0), stop=(ci == 1))
                    nc.vector.tensor_mul(
                        out=oT[0:64, j, :], in0=ps[0:64, 0:196],
                        in1=rc[0:64, acol(2 * j):acol(2 * j) + 196])
                    nc.vector.tensor_mul(
                        out=oT[64:128, j, :], in0=ps[64:128, 0:196],
                        in1=rc[64:128, acol(2 * j + 1):acol(2 * j + 1) + 196])
                else:
                    ps = ot_ps.tile([128, 392], f32, tag="ot")
                    for ci, (ko, kl) in enumerate(CH):
                        nc.tensor.matmul(
                            ps[:, :392],
                            v_sb[ci][:kl, 2 * j * 64:(2 * j + 2) * 64],
                            attn_c[ci][:kl, acol(2 * j):acol(2 * j) + 392],
                            start=(ci == 0), stop=(ci == 1))
                    nc.vector.tensor_mul(
                        out=oT[0:64, j, :], in0=ps[0:64, 0:196],
                        in1=rc[0:64, acol(2 * j):acol(2 * j) + 196])
                    nc.vector.tensor_mul(
                        out=oT[64:128, j, :], in0=ps[64:128, 196:392],
                        in1=rc[64:128, acol(2 * j + 1):acol(2 * j + 1) + 196])
                a["j"] += 1

            av_q, pj_q = [], []

            def fill_av(n):
                """Emit up to n deferred AV k-tile blocks (stage-2 filler)."""
                done = 0
                while av_q and done < n:
                    a = av_q[0]
                    if a["j"] == 0:
                        a["oT"] = work.tile([128, KT, 196], bf16, tag="oT",
                                            bufs=3, name="oT")
                    emit_av_block(a)
                    done += 1
                    if a["j"] == KT:
                        av_q.pop(0)
                        pj_q.append({"oT": a["oT"], "f": a["st"][3], "cb": 0,
                                     "i": 0, "osb": None})

            def fill_pj(n):
                """Emit up to n deferred proj chunks (stage-3 filler)."""
                done = 0
                while pj_q and done < n:
                    p = pj_q[0]
                    if p["i"] == 0:
                        p["osb"] = ost.tile([98, 2, D], f32, tag="o",
                                            name="osb")
                    emit_pj_chunk(p)
                    done += 1
                    if p["i"] == len(PROJ_CHUNKS):
                        pj_q.pop(0)

            def do_proj(st):
                oT, f, cb = st
                osb = ost.tile([98, 2, D], f32, tag="o", name="osb")
                p = {"oT": oT, "f": f, "cb": cb, "i": 0, "osb": osb}
                while p["i"] < len(PROJ_CHUNKS):
                    emit_pj_chunk(p)

            def do_av(st):
                oT = work.tile([128, KT, 196], bf16, tag="oT", bufs=3,
                                name="oT")
                a = {"st": st, "j": 0, "oT": oT}
                while a["j"] < KT:
                    emit_av_block(a)
                return (a["oT"], st[3], 0)

            pending = []
            for pair in range(NPAIR * unroll_reps):
                pair = pair % NPAIR
                frames = [f for f in (2 * pair, 2 * pair + 1) if f < n_frames]
                PW = 196 * len(frames)
                # ---- load fp32 (112-token chunks, 28-token overlap), ----
                # ---- pos-add to bf16 on GPSIMD, PE-transpose to sT/tT ----
                sT = work.tile([128, KT, 392], bf16, tag="sT")
                tT = work.tile([128, KT, 392], bf16, tag="tT")
                s16L, t16L = {}, {}
                seng = {0: nc.vector, 1: nc.gpsimd, 3: nc.gpsimd}[POOL_ADD]
                teng = {0: nc.vector, 1: nc.gpsimd, 3: nc.vector}[POOL_ADD]
                if not SKIPIO:
                    for fi, f in enumerate(frames):
                        ssb = io.tile([112, 2, D], f32, tag="s")
                        nc.sync.dma_start(
                            out=ssb,
                            in_=bass.AP(tensor=s_d.ap().tensor,
                                        offset=f * (N + 1) * D + D,
                                        ap=[[D, 112], [84 * D, 2], [1, D]]))
                        s16f = cvt.tile([112, 2, D], bf16, tag="s16")
                        tsb = io.tile([112, 2, D], f32, tag="t")
                        (nc.scalar if DMAQ else nc.sync).dma_start(
                            out=tsb,
                            in_=bass.AP(tensor=t_d.ap().tensor,
                                        offset=f * N * D,
                                        ap=[[D, 112], [84 * D, 2], [1, D]]))
                        t16f = cvt.tile([112, 2, D], bf16, tag="t16")
                        for ci in range(2):
                            seng.tensor_add(out=s16f[:, ci, :], in0=ssb[:, ci, :],
                                            in1=cpos2_sb[:, ci, :])
                            teng.tensor_add(out=t16f[:, ci, :], in0=tsb[:, ci, :],
                                            in1=vpos2_sb[:, ci, :])
                        s16L[fi] = s16f
                        t16L[fi] = t16f
                if SKIPIO:
                    for fi in range(len(frames)):
                        s16L[fi] = s16c
                        t16L[fi] = t16c

                # PE transposes (bf16): 2 chunks/frame, 28-col overlap
                # chunk A: tokens 0..111 -> cols fo..fo+112
                # chunk B: tokens 84..195 -> fo+84..fo+196 (28-col overlap
                # rewrites identical values; src must start at partition 0)
                def trans_block(dst, srcs, eng, j):
                    ps = ot_ps.tile([128, 392], bf16, tag="ot")
                    for fi in range(len(frames)):
                        for ci, co in ((0, 0), (1, 84)):
                            nc.tensor.transpose(
                                ps[:, fi * 196 + co:fi * 196 + co + 112],
                                srcs[fi][:, ci, j * 128:(j + 1) * 128],
                                ident16[:112, :112])
                    if eng == "v":
                        nc.vector.tensor_copy(out=dst[:, j, :PW],
                                              in_=ps[:, :PW])
                    else:
                        nc.scalar.copy(out=dst[:, j, :PW], in_=ps[:, :PW])

                qT = work.tile([128, KT, 392], bf16, tag="qT")
                kTt = work.tile([128, KT, 392], bf16, tag="kT")

                def gemm_block(dst, wT, src, eng, j):
                    ps = mm_ps.tile([128, 512], f32, tag="mm")
                    for kt in range(KT):
                        nc.tensor.matmul(ps[:, :PW],
                                         wT[:, kt, j * 128:(j + 1) * 128],
                                         src[:, kt, :PW],
                                         start=(kt == 0), stop=(kt == KT - 1))
                    if eng == "v":
                        nc.vector.tensor_copy(out=dst[:, j, :PW], in_=ps[:, :PW])
                    else:
                        nc.scalar.copy(out=dst[:, j, :PW], in_=ps[:, :PW])

                # sT transposes first (AV filler hides the evac bubbles),
                # then tT transposes zipped with kT GEMM blocks (kT reads
                # the completed sT), then qT GEMM blocks with proj filler.
                if SCHED == "old0":
                    # the original ordering: interleaved sT/tT transposes,
                    # deferred proj, then qT gemms, then kT gemms
                    for j in range(KT):
                        trans_block(sT, s16L, "s" if TEVAC else "v", j)
                        trans_block(tT, t16L, "s", j)
                    fill_pj(len(PROJ_CHUNKS))
                    for j in range(KT):
                        gemm_block(qT, wqT, tT, "v", j)
                    for j in range(KT):
                        gemm_block(kTt, wkT, sT, "s", j)
                elif SCHED in ("old", "old2"):
                    for j in range(KT):
                        trans_block(sT, s16L, "s" if TEVAC else "v", j)
                    for j in range(KT):
                        trans_block(tT, t16L, "s", j)
                        gemm_block(kTt, wkT, sT, "s", j)
                    fill_pj(len(PROJ_CHUNKS))
                    for j in range(KT):
                        gemm_block(qT, wqT, tT, "v", j)
                else:
                    for j in range(KT):
                        trans_block(sT, s16L, "s" if TEVAC else "v", j)
                        if j % 2 == 1:
                            fill_av(1)
                    for j in range(KT):
                        trans_block(tT, t16L, "s", j)
                        gemm_block(kTt, wkT, sT, "s", j)
                    for j in range(KT):
                        gemm_block(qT, wqT, tT, "v", j)
                        if j % 2 == 1:
                            fill_pj(1)

                if SKIPATT:
                    # ablation: proj straight from qT (wrong numerics)
                    for fi, f in enumerate(frames):
                        if SWPIPE and pending:
                            do_proj(pending.pop(0))
                        pending.append((qT, f, fi * 196))
                    continue

                for fi, f in enumerate(frames):
                    fo = fi * 196
                    # ---- QK (4-head groups) + exp, interleaved with v GEMM ----
                    attn_c = []
                    v_sb = {}
                    for ci in range(2):
                        at = work.tile([98, H * 196], bf16, tag=f"attn{fi}_{ci}", bufs=1)
                        attn_c.append(at)
                        vt = work.tile([98, D], bf16, tag=f"v{ci}")
                        v_sb[ci] = vt

                    if EXPBATCH:
                        def qk_group(ci, g):
                            # 4 heads as two 2-bank [98, 1024] psum tiles, one
                            # head per bank at col 0 (matmul PSUM writes must
                            # be bank-aligned); one exp per tile reads both
                            # banks via a bank-strided 3D AP.
                            ko, kl = CH[ci]
                            for half in range(2):
                                aps = at_ps.tile([98, 1024], f32, tag="at")
                                for hp in range(2):
                                    h = 4 * g + 2 * half + hp
                                    nc.tensor.matmul(
                                        aps[:kl, hp * 512:hp * 512 + 196],
                                        kTt[(h % 2) * 64:(h % 2) * 64 + 64, h // 2,
                                            fo + ko:fo + ko + kl],
                                        qT[(h % 2) * 64:(h % 2) * 64 + 64, h // 2,
                                           fo:fo + 196],
                                        start=True, stop=True)
                                h0 = 4 * g + 2 * half
                                nc.scalar.activation(
                                    out=view(attn_c[ci][:kl, acol(h0):acol(h0) + 392],
                                             [[196, 2], [1, 196]]),
                                    in_=view(aps[:kl, :], [[512, 2], [1, 196]]),
                                    func=EXP, scale=SCALE)
                    else:
                        def qk_group(ci, g):
                            ko, kl = CH[ci]
                            for hp in range(4):
                                h = 4 * g + hp
                                aps = at_ps.tile([98, 512], f32, tag="at")
                                nc.tensor.matmul(
                                    aps[:kl, 0:196],
                                    kTt[(h % 2) * 64:(h % 2) * 64 + 64, h // 2,
                                        fo + ko:fo + ko + kl],
                                    qT[(h % 2) * 64:(h % 2) * 64 + 64, h // 2,
                                       fo:fo + 196],
                                    start=True, stop=True)
                                nc.scalar.activation(
                                    out=attn_c[ci][:kl, h * 196:(h + 1) * 196],
                                    in_=aps[:kl, 0:196],
                                    func=EXP, scale=SCALE)

                    def v_chunk(ci, n0, nl):
                        o, l = CH[ci]
                        ps = mm_ps.tile([128, 512], f32, tag="mm")
                        for kt in range(KT):
                            nc.tensor.matmul(
                                ps[:l, :nl],
                                sT[:, kt, fo + o:fo + o + l],
                                wvT[:, kt, n0:n0 + nl],
                                start=(kt == 0), stop=(kt == KT - 1))
                        nc.scalar.copy(out=v_sb[ci][:, n0:n0 + nl],
                                       in_=ps[:l, :nl])

                    rc = rcp.tile([128, H * 196], f32, tag="rc")

                    def rowsum(n0, nl):
                        # needs exp of heads covering cols n0..n0+nl, both ci
                        ps = mm_ps.tile([128, 512], f32, tag="mm")
                        for ci, (ko, kl) in enumerate(CH):
                            nc.tensor.matmul(ps[:, :nl], ones[:kl, :],
                                             attn_c[ci][:kl, n0:n0 + nl],
                                             start=(ci == 0), stop=(ci == 1))
                        nc.vector.reciprocal_approx_fast(out=rc[:, n0:n0 + nl],
                                                         in_=ps[:, :nl])

                    # QK/exp interleaved with v GEMM, rowsums, and deferred
                    # AV(f-1) / proj(f-2) blocks so every engine has
                    # independent work while this frame's softmax chain
                    # (QK mm -> exp -> rowsum mm -> recip) propagates.
                    if not SWPIPE:
                        qk_group(0, 0)
                        v_chunk(0, *NSPLIT[0])
                        qk_group(1, 0)
                        v_chunk(0, *NSPLIT[1])
                        rowsum(*RSPLIT[0])
                        qk_group(0, 1)
                        v_chunk(1, *NSPLIT[0])
                        qk_group(1, 1)
                        v_chunk(1, *NSPLIT[1])
                        rowsum(*RSPLIT[1])
                        rowsum(*RSPLIT[2])
                        qk_group(0, 2)
                        qk_group(1, 2)
                        rowsum(*RSPLIT[3])
                        rowsum(*RSPLIT[4])
                        do_proj(do_av((attn_c, v_sb, rc, f)))
                    elif SCHED in ("old", "old0", "old2"):
                        qk_group(0, 0)
                        v_chunk(0, *NSPLIT[0])
                        qk_group(1, 0)
                        v_chunk(0, *NSPLIT[1])
                        rowsum(*RSPLIT[0])
                        qk_group(0, 1)
                        v_chunk(1, *NSPLIT[0])
                        qk_group(1, 1)
                        v_chunk(1, *NSPLIT[1])
                        rowsum(*RSPLIT[1])
                        rowsum(*RSPLIT[2])
                        if SCHED == "old2":
                            # this frame's first AV blocks are ready here
                            # (attn heads 0-5 exp'd, rc chunks 0-2 done) --
                            # they fill the PE while the last exps drain
                            av_q.append({"st": (attn_c, v_sb, rc, f), "j": 0,
                                         "oT": None})
                            qk_group(0, 2)
                            fill_av(1)
                            qk_group(1, 2)
                            fill_av(2)
                            rowsum(*RSPLIT[3])
                            rowsum(*RSPLIT[4])
                            fill_av(KT)
                            fill_pj(len(PROJ_CHUNKS))
                        else:
                            qk_group(0, 2)
                            qk_group(1, 2)
                            rowsum(*RSPLIT[3])
                            rowsum(*RSPLIT[4])
                            av_q.append({"st": (attn_c, v_sb, rc, f), "j": 0,
                                         "oT": None})
                            fill_av(KT)
                            fill_pj(len(PROJ_CHUNKS))
                    else:
                        qk_group(0, 0)
                        fill_av(2)
                        v_chunk(0, *NSPLIT[0])
                        qk_group(1, 0)
                        fill_av(2)
                        v_chunk(0, *NSPLIT[1])
                        rowsum(*RSPLIT[0])
                        qk_group(0, 1)
                        fill_av(2)
                        v_chunk(1, *NSPLIT[0])
                        qk_group(1, 1)
                        fill_av(2)
                        v_chunk(1, *NSPLIT[1])
                        rowsum(*RSPLIT[1])
                        rowsum(*RSPLIT[2])
                        qk_group(0, 2)
                        fill_pj(2)
                        qk_group(1, 2)
                        fill_pj(2)
                        rowsum(*RSPLIT[3])
                        rowsum(*RSPLIT[4])
                        fill_pj(1)
                        av_q.append({"st": (attn_c, v_sb, rc, f), "j": 0,
                                     "oT": None})

            # drain the stage queues (still inside the For_i body)
            while av_q:
                fill_av(KT)
            while pj_q:
                fill_pj(len(PROJ_CHUNKS))
            while pending:
                do_proj(pending.pop(0))

    nc.compile()
    return nc


def _get_nc(n_frames, reps=1, unroll=1):
    key = (n_frames, reps, unroll)
    if key not in _CACHE:
        _CACHE[key] = _build(n_frames, reps, unroll)
    return _CACHE[key]


def _numpy_fallback(s_x, t_x, clip_space_pos, vmae_space_pos, q_w, q_b,
                    kv_w, kv_b, proj_w, proj_b):
    Bv = t_x.shape[0]
    s_pat = s_x[:, 1:, :] + clip_space_pos
    t = t_x.reshape(Bv * T, N, D) + vmae_space_pos
    q = t @ q_w.T + q_b
    q = q.reshape(Bv * T, N, H, HD).transpose(0, 2, 1, 3)
    kv = s_pat @ kv_w.T + kv_b
    kv = kv.reshape(Bv * T, N, 2, H, HD)
    k = kv[:, :, 0].transpose(0, 2, 1, 3)
    v = kv[:, :, 1].transpose(0, 2, 1, 3)
    attn = np.einsum('bhqd,bhkd->bhqk', q * SCALE, k)
    attn = attn - attn.max(-1, keepdims=True)
    attn = np.exp(attn)
    attn = attn / attn.sum(-1, keepdims=True)
    o = np.einsum('bhqk,bhkd->bhqd', attn, v)
    o = o.transpose(0, 2, 1, 3).reshape(Bv * T, N, D)
    o = o @ proj_w.T + proj_b
    return o.reshape(Bv, T * N, D).astype(np.float32)


def _wT_host(w):
    """[rows, 768] fp32 -> [128, KT, rows] bf16 with wT[p, j, r] = w[r, j*128+p]."""
    import ml_dtypes
    wt = np.ascontiguousarray(
        w.T.reshape(KT, 128, w.shape[0]).transpose(1, 0, 2))
    return wt.astype(ml_dtypes.bfloat16)


def make_in_maps(s_x, t_x, clip_space_pos, vmae_space_pos, q_w, kv_w, proj_w):
    """Shard full inputs into the per-core input maps the NEFF expects."""
    s_x = np.ascontiguousarray(s_x, dtype=np.float32)
    t_flat = np.ascontiguousarray(t_x, dtype=np.float32).reshape(BT, N, D)
    common = {
        "cpos": np.ascontiguousarray(clip_space_pos, dtype=np.float32),
        "vpos": np.ascontiguousarray(vmae_space_pos, dtype=np.float32),
        "wqT": _wT_host(np.asarray(q_w, dtype=np.float32)),
        "wkT": _wT_host(np.asarray(kv_w, dtype=np.float32)[0:D]),
        "wvT": _wT_host(np.asarray(kv_w, dtype=np.float32)[D:2 * D]),
        "wpT": _wT_host(np.asarray(proj_w, dtype=np.float32)),
    }
    in_maps = []
    for c in range(NCORES):
        in_maps.append({
            "s": np.ascontiguousarray(s_x[c * F:(c + 1) * F]),
            "t": np.ascontiguousarray(t_flat[c * F:(c + 1) * F]),
            **common,
        })
    return in_maps


def _make_runner(nc):
    """Build a cached 8-core PJRT executor for `nc` (mirrors
    bass2jax.run_bass_via_pjrt but jits once so repeat calls skip
    NEFF reload/compile)."""
    import jax
    import concourse.mybir as mybir
    from concourse import bass2jax as b2j
    from jax.experimental.shard_map import shard_map
    from jax.sharding import Mesh, PartitionSpec

    b2j.install_neuronx_cc_hook()
    partition_name = (nc.partition_id_tensor.name
                      if nc.partition_id_tensor else None)
    in_names, out_names, out_avals, zero_outs = [], [], [], []
    for alloc in nc.m.functions[0].allocations:
        if not isinstance(alloc, mybir.MemoryLocationSet):
            continue
        name = alloc.memorylocations[0].name
        if alloc.kind == "ExternalInput":
            if name != partition_name:
                in_names.append(name)
        elif alloc.kind == "ExternalOutput":
            out_names.append(name)
            shape = tuple(alloc.tensor_shape)
            dtype = mybir.dt.np(alloc.dtype)
            out_avals.append(jax.core.ShapedArray(shape, dtype))
            zero_outs.append(np.zeros(shape, dtype))
    n_params = len(in_names)
    n_outs = len(out_avals)
    all_names = list(in_names) + list(out_names)
    if partition_name is not None:
        all_names.append(partition_name)
    donate = tuple(range(n_params, n_params + n_outs))

    def _body(*args):
        operands = list(args)
        if partition_name is not None:
            operands.append(b2j.partition_id_tensor())
        return tuple(b2j._bass_exec_p.bind(
            *operands,
            out_avals=tuple(out_avals),
            in_names=tuple(all_names),
            out_names=tuple(out_names),
            lowering_input_output_aliases=(),
            sim_require_finite=True,
            sim_require_nnan=True,
            nc=nc,
        ))

    devices = jax.devices()[:NCORES]
    mesh = Mesh(np.asarray(devices), ("core",))
    sharded = jax.jit(
        shard_map(_body, mesh=mesh,
                  in_specs=(PartitionSpec("core"),) * (n_params + n_outs),
                  out_specs=(PartitionSpec("core"),) * n_outs,
                  check_rep=False),
        donate_argnums=donate, keep_unused=True)

    def prep(in_maps):
        return [np.concatenate([np.asarray(m[name]) for m in in_maps],
                               axis=0) for name in in_names]

    def mkzeros():
        return [np.zeros((NCORES * z.shape[0], *z.shape[1:]), z.dtype)
                for z in zero_outs]

    def run(in_maps):
        outs = sharded(*prep(in_maps), *mkzeros())
        return {name: np.asarray(outs[i]) for i, name in enumerate(out_names)}

    run.sharded = sharded
    run.prep = prep
    run.mkzeros = mkzeros
    run.out_names = out_names
    return run


def _get_runner(n_frames):
    key = ("runner", n_frames)
    if key not in _CACHE:
        _CACHE[key] = _make_runner(_get_nc(n_frames))
    return _CACHE[key]


def kernel(s_x, t_x, clip_space_pos, vmae_space_pos, q_w, q_b, kv_w, kv_b,
           proj_w, proj_b):
    if np.any(q_b) or np.any(kv_b) or np.any(proj_b):
        # biases are spec'd zero; exact CPU path keeps the contract if not
        return _numpy_fallback(s_x, t_x, clip_space_pos, vmae_space_pos,
                               q_w, q_b, kv_w, kv_b, proj_w, proj_b)

    in_maps = make_in_maps(s_x, t_x, clip_space_pos, vmae_space_pos,
                           q_w, kv_w, proj_w)
    run = _get_runner(F)
    out = run(in_maps)["out"]
    return out.reshape(B, T * N, D)


if __name__ == "__main__":
    rng = np.random.default_rng(0)
    ins = {
        "s_x": rng.standard_normal((BT, N + 1, D), dtype=np.float32),
        "t_x": rng.standard_normal((B, T * N, D), dtype=np.float32),
        "clip_space_pos": SCALE * rng.standard_normal((N, D), dtype=np.float32),
        "vmae_space_pos": SCALE * rng.standard_normal((N, D), dtype=np.float32),
        "q_w": (0.02 * rng.standard_normal((D, D))).astype(np.float32),
        "q_b": np.zeros(D, np.float32),
        "kv_w": (0.02 * rng.standard_normal((2 * D, D))).astype(np.float32),
        "kv_b": np.zeros(2 * D, np.float32),
        "proj_w": (0.02 * rng.standard_normal((D, D))).astype(np.float32),
        "proj_b": np.zeros(D, np.float32),
    }
    got = kernel(**ins)
    ref = _numpy_fallback(**ins)
    err = np.abs(got - ref)
    scale = np.abs(ref).max()
    print(f"abs_max_err={err.max():.3e}  rel_to_scale={err.max()/scale:.3e} "
          f"mean={err.mean():.3e}")
